# revision 22
# baseline (speedup 1.0000x reference)
"""Trainium2 Bass kernel for nn_GTCNN (product-graph GTCNN, 2 layers, K collapsed).

Math (per batch b, x: [M=8192, 32]):
  Adj = s0*I + s1*kron(I_t, As) + s2*kron(At, I_s) + s3*kron(At, As),  T=64, N=128
  h0 = x @ W1 + b1 ; h_{l+1} = tanh((Adj h_l) Heff_l) ; out = h2 @ W2 + b2

v3 device dataflow, per core = (b, t-quarter q); indices n = 32a+i, t = 32c+u, h:
  layouts:
    FD  [part (a,h), free (t,i)]    - feature-contraction matmuls (kron(I4, .))
    NM  [part (a,i), free]          - P/Q n-mix matmuls
    FDT [part (a,u), free (c,i,h)]  - At t-mix matmuls (accumulate over c)
  L1 folds W1 and Heff0 on host:  w = x @ (W1 Heff0)  [FD]
     zpre1 = P w + Q (At w) + rs (x) b~   with b~ = b1 Heff0 and
     rs[(t,n)] = rowsum(P)[n] + rowsum(Q)[n] rowsum(At)[t]  (bias mixed by Adj),
     implemented as a third accumulating matmul with stationary rs.T [64,128]
     and a constant moving delta(t) (x) b~ [64, 2048].
     tanh -> z1 [NM bf16, phys (h,t)]
  L2: P2 native on z1 quarter; A2 z1->FDT; At2 (quarter, padded); C2/D2 small
     fp32 transposes; H1/Q2 run plain fp32; tanh -> z2 [FD quarter]; W2 + b2.
  All big matmuls bf16 (1 cyc/row); PSUM->bf16 casts for the two big transpose
  sources (w, u1) ride the scalar engine. Host packs x / unpacks out, so no
  on-device transposes are spent on I/O.
"""

import numpy as np

T, NS, B, FIN, HID, FOUT = 64, 128, 2, 32, 32, 16
M = T * NS
NCORES, NQ = 8, 4
TQ = T // NQ  # 16 t's per quarter

_CACHE = {}


def _build_nc():
    from contextlib import ExitStack

    import concourse.mybir as mybir
    import concourse.tile as tile
    from concourse import bacc
    from concourse.bass import ds

    f32 = mybir.dt.float32
    bf = mybir.dt.bfloat16
    AF = mybir.ActivationFunctionType

    nc = bacc.Bacc(
        "TRN2",
        target_bir_lowering=False,
        debug=False,
        enable_asserts=False,
        num_devices=NCORES,
    )

    # ---- DRAM I/O ----
    xp = nc.dram_tensor("xp", [128, 2048], bf, kind="ExternalInput")  # [(a,fin),(t,i)]
    # bf16 stationaries, p-major [128, 12*128]; slot s at cols [128s, 128s+128):
    # 0:wti4 (W1@Heff0 kron) 1:B1S (rs.T, rows 0..63) 2:- 3:w2i4 4:P 5:Q
    # 6..9:atb[c][cp] at 6+2c+cp  10,11:atq (per-core quarter)
    stat = nc.dram_tensor("stat", [128, 12 * 128], bf, kind="ExternalInput")
    dlb = nc.dram_tensor("dlb", [64, 2048], bf, kind="ExternalInput")  # delta(t)xb~
    bias = nc.dram_tensor("bias", [128, 1], f32, kind="ExternalInput")  # b2 tiled
    outb = nc.dram_tensor("outb", [128, 512], bf, kind="ExternalOutput")

    H1024 = [slice(1024 * j, 1024 * (j + 1)) for j in range(2)]
    C512 = [slice(512 * j, 512 * (j + 1)) for j in range(4)]

    with tile.TileContext(nc) as tc, ExitStack() as ctx:
        const = ctx.enter_context(tc.tile_pool(name="const", bufs=1))
        st = ctx.enter_context(tc.tile_pool(name="st", bufs=1))
        ps = ctx.enter_context(tc.tile_pool(name="ps", bufs=4, space="PSUM"))

        pid = nc.tensor.partition_id()
        toff = (pid % NQ) * TQ  # t-offset of this core's quarter
        vq = (pid % 2) * TQ  # u'-offset within the padded 32-block

        # ---- constants (scalar HWDGE queue, parallel to x on sync) ----
        stat_f = const.tile([128, 12 * 128], bf, tag="stat")
        nc.scalar.dma_start(stat_f[:], stat.ap())
        dlb_s = const.tile([64, 2048], bf, tag="dlb")
        nc.scalar.dma_start(dlb_s[:], dlb.ap())
        bias_s = const.tile([128, 1], f32, tag="bias")
        nc.scalar.dma_start(bias_s[:], bias.ap())
        stat_s = stat_f[:].rearrange("p (s c) -> p s c", s=12, c=128)
        wti4 = stat_s[:, 0, :]
        b1S = stat_s[0:64, 1, :]
        w2i4 = stat_s[:, 3, :]
        pmat = stat_s[:, 4, :]
        qmat = stat_s[:, 5, :]
        atb = [[stat_s[:, 6 + 2 * c + cp, :] for cp in range(2)] for c in range(2)]
        atq = [stat_s[:, 10 + c, :] for c in range(2)]
        hf1 = stat_s[:, 2, :]

        # ---- x load (packed on host): 4 chunks ----
        x_s = st.tile([128, 2048], bf, tag="x")
        for j in range(4):
            nc.sync.dma_start(x_s[:, C512[j]], xp.ap()[:, C512[j]])

        # =========================== layer 1 ===========================
        # w = x @ (W1 Heff0)   [FD psum, phys (t,i)]
        wp = [ps.tile([128, 1024], f32, tag="ps", name=f"wp{j}") for j in range(2)]
        for j in range(4):
            nc.tensor.matmul(wp[j // 2][:, C512[j % 2]], wti4, x_s[:, C512[j]],
                             start=True, stop=True)
        # cast to bf16 (scalar engine), per c-half; wb phys (c, u, i)
        wb = st.tile([128, 2048], bf, tag="wb")
        for c in range(2):
            nc.scalar.activation(wb[:, H1024[c]], wp[c][:], AF.Identity)

        # D1a: w -> NM   w_nm phys (t, h); contiguous 32-runs on both sides
        w_nm = st.tile([128, 2048], bf, tag="w_nm")
        w_nm_v = w_nm[:].rearrange("p (t h) -> p t h", t=64, h=32)
        wb_v = wb[:].rearrange("p (c u i) -> p c u i", c=2, u=32, i=32)
        for c in range(2):
            nc.vector.transpose(
                out=w_nm_v[:, 32 * c:32 * (c + 1), :], in_=wb_v[:, c]
            )
        # D1b: w -> FDT  w_fdt phys (c, i, h); in iterated (i, u-run str 32)
        w_fdt = st.tile([128, 2048], bf, tag="w_fdt")
        w_fdt_v = w_fdt[:].rearrange("p (c i h) -> p c i h", c=2, i=32, h=32)
        wb_t = wb[:].rearrange("p (c u i) -> p c i u", c=2, u=32, i=32)
        for c in range(2):
            nc.vector.transpose(out=w_fdt_v[:, c], in_=wb_t[:, c])

        # At1: u1[(a,u'), (c', i, h)] accumulated over c; 2 psum tiles by c'
        u1p = [ps.tile([128, 1024], f32, tag="ps", name=f"u1p{j}") for j in range(2)]
        for cp in range(2):
            for c in range(2):
                for k in range(2):
                    nc.tensor.matmul(
                        u1p[cp][:, 512 * k:512 * (k + 1)],
                        atb[c][cp],
                        w_fdt[:, 1024 * c + 512 * k:1024 * c + 512 * (k + 1)],
                        start=(c == 0),
                        stop=(c == 1),
                    )
        # cast u1 to bf16 (scalar engine), per c'
        u1b = st.tile([128, 2048], bf, tag="u1b")
        for cp in range(2):
            nc.scalar.activation(u1b[:, H1024[cp]], u1p[cp][:], AF.Identity)

        # C1: u1 -> NM   u_nm phys (h, c', u'); in per c' iterated (h, i-run)
        u_nm = st.tile([128, 2048], bf, tag="u_nm")
        u_nm_v = u_nm[:].rearrange("p (h c u) -> p c h u", c=2, u=32, h=32)
        u1b_v = u1b[:].rearrange("p (c i h) -> p c h i", c=2, i=32, h=32)
        for cp in range(2):
            nc.vector.transpose(out=u_nm_v[:, cp], in_=u1b_v[:, cp])

        # zpre1 = rs (x) b~  +  P w + Q u   (NM psum, phys (h, t))
        zp1 = [ps.tile([128, 1024], f32, tag="ps", name=f"zp1{j}") for j in range(2)]
        for j in range(4):
            nc.tensor.matmul(zp1[j // 2][:, C512[j % 2]], b1S, dlb_s[:, C512[j]],
                             start=True, stop=False)
        w_nm_s = w_nm[:].rearrange("p (c u h) -> p h c u", c=2, u=32, h=32)
        for j in range(4):
            nc.tensor.matmul(zp1[j // 2][:, C512[j % 2]], pmat,
                             w_nm_s[:, 8 * j:8 * (j + 1)], start=False, stop=False)
        for j in range(4):
            nc.tensor.matmul(zp1[j // 2][:, C512[j % 2]], qmat, u_nm[:, C512[j]],
                             start=False, stop=True)

        # tanh -> z1 [NM bf16, phys (h, t)]
        z1 = st.tile([128, 2048], bf, tag="z1")
        for hh in range(2):
            nc.scalar.activation(z1[:, H1024[hh]], zp1[hh][:], AF.Tanh)

        # =========================== layer 2 ===========================
        # A2: z1 -> FDT  g2 phys (c, h, i); in per c (h, u-run)
        g2 = st.tile([128, 2048], bf, tag="g2")
        g2_v = g2[:].rearrange("p (c h i) -> p c h i", c=2, h=32, i=32)
        z1_v = z1[:].rearrange("p (h c u) -> p c h u", c=2, u=32, h=32)
        for c in range(2):
            nc.vector.transpose(out=g2_v[:, c], in_=z1_v[:, c])

        # At2 (quarter rows, padded to u'32): accumulate over c; moving per c (h,i)
        u2p = ps.tile([128, 1024], f32, tag="ps")
        for c in range(2):
            for k in range(2):
                nc.tensor.matmul(
                    u2p[:, 512 * k:512 * (k + 1)],
                    atq[c],
                    g2[:, 1024 * c + 512 * k:1024 * c + 512 * (k + 1)],
                    start=(c == 0),
                    stop=(c == 1),
                )

        # cast u2 to bf16; C2: u2 -> NM  u2_nm phys (h, u'32)
        u2b = st.tile([128, 1024], bf, tag="u2b")
        nc.scalar.activation(u2b[:], u2p[:], AF.Identity)
        u2_nm = st.tile([128, 1024], bf, tag="u2_nm")
        nc.vector.transpose(
            out=u2_nm[:].rearrange("p (h u) -> p h u", h=32, u=32),
            in_=u2b[:].rearrange("p (h i) -> p h i", h=32, i=32),
        )

        # zpre2 = P z1[quarter] (bf16) + Q u2 (fp32)  -> psum (v, h) stream
        zp2 = ps.tile([128, 512], f32, tag="ps")
        z1_t = z1[:].rearrange("p (h t) -> p t h", h=32, t=64)
        nc.tensor.matmul(zp2[:], pmat, z1_t[:, ds(toff, TQ), :], start=True, stop=False)
        u2_t = u2_nm[:].rearrange("p (h u) -> p u h", h=32, u=32)
        nc.tensor.matmul(zp2[:], qmat, u2_t[:, ds(vq, TQ), :],
                         start=False, stop=True)

        # cast zpre2 to bf16; D2: -> FD quarter  z2f phys (v, i)
        zp2b = st.tile([128, 512], bf, tag="zp2b")
        nc.scalar.activation(zp2b[:], zp2[:], AF.Identity)
        z2f = st.tile([128, 512], bf, tag="z2f")
        nc.vector.transpose(
            out=z2f[:].rearrange("p (v i) -> p v i", v=16, i=32),
            in_=zp2b[:].rearrange("p (v h) -> p v h", v=16, h=32),
        )

        # pre2 = z2f @ Heff1 ; tanh
        p2p = ps.tile([128, 512], f32, tag="ps")
        nc.tensor.matmul(p2p[:], hf1, z2f[:], start=True, stop=True)
        z2 = st.tile([128, 512], bf, tag="z2")
        nc.scalar.activation(z2[:], p2p[:], AF.Tanh)

        # out = z2 @ W2 + b2  (FD quarter)
        op = ps.tile([128, 512], f32, tag="ps")
        nc.tensor.matmul(op[:], w2i4, z2[:], start=True, stop=True)
        o_s = st.tile([128, 512], bf, tag="o")
        nc.scalar.activation(o_s[:], op[:], AF.Identity, bias=bias_s[:, 0:1])

        nc.sync.dma_start(outb.ap(), o_s[:])

    nc.compile()
    return nc


def _host_weights(Adj_t, Adj_s, s, H, W1, b1, W2, b2):
    import ml_dtypes

    bfp = ml_dtypes.bfloat16
    f4 = np.float32
    I4 = np.eye(4, dtype=f4)
    I128 = np.eye(128, dtype=f4)
    Heff = H.sum(axis=1).astype(f4)  # [2, 32, 32]

    P = (s[0] * I128 + s[1] * Adj_s).astype(f4)
    Q = (s[2] * I128 + s[3] * Adj_s).astype(f4)

    def k4(m):
        return np.kron(I4, m.astype(f4))

    w2pad = np.zeros((32, 32), dtype=f4)
    w2pad[:, :FOUT] = W2

    wt = (W1 @ Heff[0]).astype(f4)  # folded W1*Heff0
    bt = (b1 @ Heff[0]).astype(f4)  # folded bias [32]
    rsP = P.sum(axis=1)  # [128]
    rsQ = Q.sum(axis=1)
    rsAt = Adj_t.sum(axis=1).astype(f4)  # [64]
    rs = rsP[None, :] + rsAt[:, None] * rsQ[None, :]  # [64 t, 128 n]

    stat = np.zeros((12, 128, 128), dtype=f4)
    stat[0] = k4(wt)
    stat[1, :64, :] = rs  # B1S[t2, n]
    stat[2] = k4(Heff[1])
    stat[3] = k4(w2pad)
    stat[4] = P
    stat[5] = Q
    for c in range(2):
        for cp in range(2):
            stat[6 + 2 * c + cp] = k4(Adj_t[32 * c:32 * c + 32, 32 * cp:32 * cp + 32])

    stat_all = np.broadcast_to(stat, (NQ, 12, 128, 128)).copy()
    for q in range(NQ):
        cq, vq = q // 2, (q % 2) * 16
        for c in range(2):
            blk = np.zeros((32, 32), dtype=f4)
            blk[:, vq:vq + 16] = Adj_t[32 * c:32 * c + 32, 32 * cq + vq:32 * cq + vq + 16]
            stat_all[q, 10 + c] = k4(blk)

    # delta(t) x b~ moving const, (h, t)-stream: dlb[t2, h*64 + t] = (t==t2)*bt[h]
    dlbm = np.zeros((64, 32, 64), dtype=f4)
    for t2 in range(64):
        dlbm[t2, :, t2] = bt
    dlbm = dlbm.reshape(64, 2048)

    b2pad = np.zeros(32, dtype=f4)
    b2pad[:FOUT] = b2
    bias = np.ascontiguousarray(np.tile(b2pad, 4)[:, None].astype(f4))

    stat_pm = np.ascontiguousarray(
        stat_all.transpose(0, 2, 1, 3).reshape(NQ, 128, 12 * 128).astype(bfp))
    return stat_pm, np.ascontiguousarray(dlbm.astype(bfp)), bias


def _in_maps(inputs):
    import ml_dtypes

    bfp = ml_dtypes.bfloat16
    f4 = np.float32
    x = np.asarray(inputs["x"], dtype=f4)
    stat_pm, dlbm, bias = _host_weights(
        np.asarray(inputs["Adj_t"], dtype=f4),
        np.asarray(inputs["Adj_s"], dtype=f4),
        np.asarray(inputs["s"], dtype=f4),
        np.asarray(inputs["H"], dtype=f4),
        np.asarray(inputs["W1"], dtype=f4),
        np.asarray(inputs["b1"], dtype=f4),
        np.asarray(inputs["W2"], dtype=f4),
        np.asarray(inputs["b2"], dtype=f4),
    )
    # x pack: [t, a, i, fin] -> [(a,fin), (t,i)]
    xs = []
    for b in range(B):
        xb = x[b].reshape(T, 4, 32, FIN).transpose(1, 3, 0, 2).reshape(128, T * 32)
        xs.append(np.ascontiguousarray(xb.astype(bfp)))
    maps = []
    for c in range(NCORES):
        b, q = c // NQ, c % NQ
        maps.append({
            "xp": xs[b],
            "stat": stat_pm[q],
            "dlb": dlbm,
            "bias": bias,
        })
    return maps


def _unpack_out(res_outb):
    """outb [128=(a,f32), 512=(v,i)] bf16 -> [2048, 16] fp32 for that quarter."""
    o = np.asarray(res_outb).astype(np.float32).reshape(4, 32, 16, 32)  # [a, f, v, i]
    return o.transpose(2, 0, 3, 1).reshape(16 * 128, 32)[:, :FOUT]


def kernel(**inputs) -> np.ndarray:
    import os

    from concourse import bass_utils

    if "nc" not in _CACHE:
        _CACHE["nc"] = _build_nc()
    nc = _CACHE["nc"]

    maps = _in_maps(inputs)
    trace = bool(int(os.environ.get("GTCNN_TRACE", "0")))
    res = bass_utils.run_bass_kernel_spmd(
        nc,
        maps,
        core_ids=list(range(NCORES)),
        trace=trace,
        trace_cores=list(range(NCORES)) if trace else None,
        stitch_traces=False,
    )
    _CACHE["last_results"] = res

    out = np.empty((B, M, FOUT), dtype=np.float32)
    for c in range(NCORES):
        b, q = c // NQ, c % NQ
        out[b, 2048 * q:2048 * (q + 1), :] = _unpack_out(res.results[c]["outb"])
    return out


# revision 23
# speedup vs baseline: 1.0996x; 1.0996x over previous
"""Trainium2 Bass kernel for nn_GTCNN (product-graph GTCNN, 2 layers, K collapsed).

Math (per batch b, x: [M=8192, 32]):
  Adj = s0*I + s1*kron(I_t, As) + s2*kron(At, I_s) + s3*kron(At, As),  T=64, N=128
  h0 = x @ W1 + b1 ; h_{l+1} = tanh((Adj h_l) Heff_l) ; out = h2 @ W2 + b2

v3 device dataflow, per core = (b, t-quarter q); indices n = 32a+i, t = 32c+u, h:
  layouts:
    FD  [part (a,h), free (t,i)]    - feature-contraction matmuls (kron(I4, .))
    NM  [part (a,i), free]          - P/Q n-mix matmuls
    FDT [part (a,u), free (c,i,h)]  - At t-mix matmuls (accumulate over c)
  L1 folds W1 and Heff0 on host:  w = x @ (W1 Heff0)  [FD]
     zpre1 = P w + Q (At w) + rs (x) b~   with b~ = b1 Heff0 and
     rs[(t,n)] = rowsum(P)[n] + rowsum(Q)[n] rowsum(At)[t]  (bias mixed by Adj),
     implemented as a third accumulating matmul with stationary rs.T [64,128]
     and a constant moving delta(t) (x) b~ [64, 2048].
     tanh -> z1 [NM bf16, phys (h,t)]
  L2: P2 native on z1 quarter; A2 z1->FDT; At2 (quarter, padded); C2/D2 small
     fp32 transposes; H1/Q2 run plain fp32; tanh -> z2 [FD quarter]; W2 + b2.
  All big matmuls bf16 (1 cyc/row); PSUM->bf16 casts for the two big transpose
  sources (w, u1) ride the scalar engine. Host packs x / unpacks out, so no
  on-device transposes are spent on I/O.
"""

import numpy as np

T, NS, B, FIN, HID, FOUT = 64, 128, 2, 32, 32, 16
M = T * NS
NCORES, NQ = 8, 4
TQ = T // NQ  # 16 t's per quarter

_CACHE = {}


def _build_nc():
    from contextlib import ExitStack

    import concourse.mybir as mybir
    import concourse.tile as tile
    from concourse import bacc
    from concourse.bass import ds

    f32 = mybir.dt.float32
    bf = mybir.dt.bfloat16
    AF = mybir.ActivationFunctionType

    nc = bacc.Bacc(
        "TRN2",
        target_bir_lowering=False,
        debug=False,
        enable_asserts=False,
        num_devices=NCORES,
    )

    # ---- DRAM I/O ----
    xp = nc.dram_tensor("xp", [128, 2048], bf, kind="ExternalInput")  # [(a,fin),(t,i)]
    # bf16 stationaries, p-major [128, 12*128]; slot s at cols [128s, 128s+128):
    # 0:wti4 (W1@Heff0 kron) 1:B1S (rs.T, rows 0..63) 2:- 3:w2i4 4:P 5:Q
    # 6..9:atb[c][cp] at 6+2c+cp  10,11:atq (per-core quarter)
    stat = nc.dram_tensor("stat", [128, 12 * 128], bf, kind="ExternalInput")
    dlb = nc.dram_tensor("dlb", [64, 2048], bf, kind="ExternalInput")  # delta(t)xb~
    bias = nc.dram_tensor("bias", [128, 1], f32, kind="ExternalInput")  # b2 tiled
    outb = nc.dram_tensor("outb", [128, 512], bf, kind="ExternalOutput")

    H1024 = [slice(1024 * j, 1024 * (j + 1)) for j in range(2)]
    C512 = [slice(512 * j, 512 * (j + 1)) for j in range(4)]

    with tile.TileContext(nc) as tc, ExitStack() as ctx:
        const = ctx.enter_context(tc.tile_pool(name="const", bufs=1))
        st = ctx.enter_context(tc.tile_pool(name="st", bufs=1))
        ps = ctx.enter_context(tc.tile_pool(name="ps", bufs=4, space="PSUM"))

        # ---- constants (scalar HWDGE queue, parallel to x on sync) ----
        stat_f = const.tile([128, 12 * 128], bf, tag="stat")
        nc.scalar.dma_start(stat_f[:], stat.ap())
        dlb_s = const.tile([64, 2048], bf, tag="dlb")
        nc.scalar.dma_start(dlb_s[:], dlb.ap())
        bias_s = const.tile([128, 1], f32, tag="bias")
        nc.scalar.dma_start(bias_s[:], bias.ap())
        stat_s = stat_f[:].rearrange("p (s c) -> p s c", s=12, c=128)
        wti4 = stat_s[:, 0, :]
        b1S = stat_s[0:64, 1, :]
        w2i4 = stat_s[:, 3, :]
        pmat = stat_s[:, 4, :]
        qmat = stat_s[:, 5, :]
        atb = [[stat_s[:, 6 + 2 * c + cp, :] for cp in range(2)] for c in range(2)]
        atq = [stat_s[:, 10 + c, :] for c in range(2)]
        hf1 = stat_s[:, 2, :]

        # ---- PE warm-up: ~4us of dummy matmuls on constants so the HAM
        # clock gate opens (1.2 -> 2.4 GHz) before the real chain starts ----
        warm = ps.tile([128, 512], f32, tag="ps", name="warm")
        for _k in range(10):
            nc.tensor.matmul(warm[:], wti4, stat_f[:, 0:512], start=True, stop=True)

        # ---- x load (packed on host): 4 chunks ----
        x_s = st.tile([128, 2048], bf, tag="x")
        for j in range(4):
            nc.sync.dma_start(x_s[:, C512[j]], xp.ap()[:, C512[j]])

        # =========================== layer 1 ===========================
        # w = x @ (W1 Heff0)   [FD psum, phys (t,i)]
        wp = [ps.tile([128, 1024], f32, tag="ps", name=f"wp{j}") for j in range(2)]
        for j in range(4):
            nc.tensor.matmul(wp[j // 2][:, C512[j % 2]], wti4, x_s[:, C512[j]],
                             start=True, stop=True)
        # cast to bf16 (scalar engine), per c-half; wb phys (c, u, i)
        wb = st.tile([128, 2048], bf, tag="wb")
        for c in range(2):
            nc.scalar.activation(wb[:, H1024[c]], wp[c][:], AF.Identity)

        # D1a: w -> NM   w_nm phys (t, h); contiguous 32-runs on both sides
        w_nm = st.tile([128, 2048], bf, tag="w_nm")
        w_nm_v = w_nm[:].rearrange("p (t h) -> p t h", t=64, h=32)
        wb_v = wb[:].rearrange("p (c u i) -> p c u i", c=2, u=32, i=32)
        for c in range(2):
            nc.vector.transpose(
                out=w_nm_v[:, 32 * c:32 * (c + 1), :], in_=wb_v[:, c]
            )
        # D1b: w -> FDT  w_fdt phys (c, i, h); in iterated (i, u-run str 32)
        w_fdt = st.tile([128, 2048], bf, tag="w_fdt")
        w_fdt_v = w_fdt[:].rearrange("p (c i h) -> p c i h", c=2, i=32, h=32)
        wb_t = wb[:].rearrange("p (c u i) -> p c i u", c=2, u=32, i=32)
        for c in range(2):
            nc.vector.transpose(out=w_fdt_v[:, c], in_=wb_t[:, c])

        # At1: u1[(a,u'), (c', i, h)] accumulated over c; 2 psum tiles by c'
        u1p = [ps.tile([128, 1024], f32, tag="ps", name=f"u1p{j}") for j in range(2)]
        for cp in range(2):
            for c in range(2):
                for k in range(2):
                    nc.tensor.matmul(
                        u1p[cp][:, 512 * k:512 * (k + 1)],
                        atb[c][cp],
                        w_fdt[:, 1024 * c + 512 * k:1024 * c + 512 * (k + 1)],
                        start=(c == 0),
                        stop=(c == 1),
                    )
        # cast u1 to bf16 (scalar engine), per c'
        u1b = st.tile([128, 2048], bf, tag="u1b")
        for cp in range(2):
            nc.scalar.activation(u1b[:, H1024[cp]], u1p[cp][:], AF.Identity)

        # C1: u1 -> NM   u_nm phys (h, c', u'); in per c' iterated (h, i-run)
        u_nm = st.tile([128, 2048], bf, tag="u_nm")
        u_nm_v = u_nm[:].rearrange("p (h c u) -> p c h u", c=2, u=32, h=32)
        u1b_v = u1b[:].rearrange("p (c i h) -> p c h i", c=2, i=32, h=32)
        for cp in range(2):
            nc.vector.transpose(out=u_nm_v[:, cp], in_=u1b_v[:, cp])

        # zpre1 = rs (x) b~  +  P w + Q u   (NM psum, phys (h, t))
        zp1 = [ps.tile([128, 1024], f32, tag="ps", name=f"zp1{j}") for j in range(2)]
        for j in range(4):
            nc.tensor.matmul(zp1[j // 2][:, C512[j % 2]], b1S, dlb_s[:, C512[j]],
                             start=True, stop=False)
        w_nm_s = w_nm[:].rearrange("p (c u h) -> p h c u", c=2, u=32, h=32)
        for j in range(4):
            nc.tensor.matmul(zp1[j // 2][:, C512[j % 2]], pmat,
                             w_nm_s[:, 8 * j:8 * (j + 1)], start=False, stop=False)
        for j in range(4):
            nc.tensor.matmul(zp1[j // 2][:, C512[j % 2]], qmat, u_nm[:, C512[j]],
                             start=False, stop=True)

        # tanh -> z1 [NM bf16, phys (h, t)]
        z1 = st.tile([128, 2048], bf, tag="z1")
        for hh in range(2):
            nc.scalar.activation(z1[:, H1024[hh]], zp1[hh][:], AF.Tanh)

        # =========================== layer 2 ===========================
        # A2: z1 -> FDT  g2 phys (c, h, i); in per c (h, u-run)
        g2 = st.tile([128, 2048], bf, tag="g2")
        g2_v = g2[:].rearrange("p (c h i) -> p c h i", c=2, h=32, i=32)
        z1_v = z1[:].rearrange("p (h c u) -> p c h u", c=2, u=32, h=32)
        for c in range(2):
            nc.vector.transpose(out=g2_v[:, c], in_=z1_v[:, c])

        # At2 (quarter rows, padded to u'32): accumulate over c; moving per c (h,i)
        u2p = ps.tile([128, 1024], f32, tag="ps")
        for c in range(2):
            for k in range(2):
                nc.tensor.matmul(
                    u2p[:, 512 * k:512 * (k + 1)],
                    atq[c],
                    g2[:, 1024 * c + 512 * k:1024 * c + 512 * (k + 1)],
                    start=(c == 0),
                    stop=(c == 1),
                )

        # cast u2 to bf16; C2: u2 -> NM  u2_nm phys (h, u'32)
        u2b = st.tile([128, 1024], bf, tag="u2b")
        nc.scalar.activation(u2b[:], u2p[:], AF.Identity)
        u2_nm = st.tile([128, 1024], bf, tag="u2_nm")
        nc.vector.transpose(
            out=u2_nm[:].rearrange("p (h u) -> p h u", h=32, u=32),
            in_=u2b[:].rearrange("p (h i) -> p h i", h=32, i=32),
        )

        # zpre2 = P z1[quarter] + Q u2  -> psum (v, h) stream
        pid = nc.tensor.partition_id()
        toff = (pid % NQ) * TQ  # t-offset of this core's quarter
        vq = (pid % 2) * TQ  # u'-offset within the padded 32-block
        zp2 = ps.tile([128, 512], f32, tag="ps")
        z1_t = z1[:].rearrange("p (h t) -> p t h", h=32, t=64)
        nc.tensor.matmul(zp2[:], pmat, z1_t[:, ds(toff, TQ), :], start=True, stop=False)
        u2_t = u2_nm[:].rearrange("p (h u) -> p u h", h=32, u=32)
        nc.tensor.matmul(zp2[:], qmat, u2_t[:, ds(vq, TQ), :],
                         start=False, stop=True)

        # cast zpre2 to bf16; D2: -> FD quarter  z2f phys (v, i)
        zp2b = st.tile([128, 512], bf, tag="zp2b")
        nc.scalar.activation(zp2b[:], zp2[:], AF.Identity)
        z2f = st.tile([128, 512], bf, tag="z2f")
        nc.vector.transpose(
            out=z2f[:].rearrange("p (v i) -> p v i", v=16, i=32),
            in_=zp2b[:].rearrange("p (v h) -> p v h", v=16, h=32),
        )

        # pre2 = z2f @ Heff1 ; tanh
        p2p = ps.tile([128, 512], f32, tag="ps")
        nc.tensor.matmul(p2p[:], hf1, z2f[:], start=True, stop=True)
        z2 = st.tile([128, 512], bf, tag="z2")
        nc.scalar.activation(z2[:], p2p[:], AF.Tanh)

        # out = z2 @ W2 + b2  (FD quarter)
        op = ps.tile([128, 512], f32, tag="ps")
        nc.tensor.matmul(op[:], w2i4, z2[:], start=True, stop=True)
        o_s = st.tile([128, 512], bf, tag="o")
        nc.scalar.activation(o_s[:], op[:], AF.Identity, bias=bias_s[:, 0:1])

        nc.sync.dma_start(outb.ap(), o_s[:])

    nc.compile()
    return nc


def _host_weights(Adj_t, Adj_s, s, H, W1, b1, W2, b2):
    import ml_dtypes

    bfp = ml_dtypes.bfloat16
    f4 = np.float32
    I4 = np.eye(4, dtype=f4)
    I128 = np.eye(128, dtype=f4)
    Heff = H.sum(axis=1).astype(f4)  # [2, 32, 32]

    P = (s[0] * I128 + s[1] * Adj_s).astype(f4)
    Q = (s[2] * I128 + s[3] * Adj_s).astype(f4)

    def k4(m):
        return np.kron(I4, m.astype(f4))

    w2pad = np.zeros((32, 32), dtype=f4)
    w2pad[:, :FOUT] = W2

    wt = (W1 @ Heff[0]).astype(f4)  # folded W1*Heff0
    bt = (b1 @ Heff[0]).astype(f4)  # folded bias [32]
    rsP = P.sum(axis=1)  # [128]
    rsQ = Q.sum(axis=1)
    rsAt = Adj_t.sum(axis=1).astype(f4)  # [64]
    rs = rsP[None, :] + rsAt[:, None] * rsQ[None, :]  # [64 t, 128 n]

    stat = np.zeros((12, 128, 128), dtype=f4)
    stat[0] = k4(wt)
    stat[1, :64, :] = rs  # B1S[t2, n]
    stat[2] = k4(Heff[1])
    stat[3] = k4(w2pad)
    stat[4] = P
    stat[5] = Q
    for c in range(2):
        for cp in range(2):
            stat[6 + 2 * c + cp] = k4(Adj_t[32 * c:32 * c + 32, 32 * cp:32 * cp + 32])

    stat_all = np.broadcast_to(stat, (NQ, 12, 128, 128)).copy()
    for q in range(NQ):
        cq, vq = q // 2, (q % 2) * 16
        for c in range(2):
            blk = np.zeros((32, 32), dtype=f4)
            blk[:, vq:vq + 16] = Adj_t[32 * c:32 * c + 32, 32 * cq + vq:32 * cq + vq + 16]
            stat_all[q, 10 + c] = k4(blk)

    # delta(t) x b~ moving const, (h, t)-stream: dlb[t2, h*64 + t] = (t==t2)*bt[h]
    dlbm = np.zeros((64, 32, 64), dtype=f4)
    for t2 in range(64):
        dlbm[t2, :, t2] = bt
    dlbm = dlbm.reshape(64, 2048)

    b2pad = np.zeros(32, dtype=f4)
    b2pad[:FOUT] = b2
    bias = np.ascontiguousarray(np.tile(b2pad, 4)[:, None].astype(f4))

    stat_pm = np.ascontiguousarray(
        stat_all.transpose(0, 2, 1, 3).reshape(NQ, 128, 12 * 128).astype(bfp))
    return stat_pm, np.ascontiguousarray(dlbm.astype(bfp)), bias


def _in_maps(inputs):
    import ml_dtypes

    bfp = ml_dtypes.bfloat16
    f4 = np.float32
    x = np.asarray(inputs["x"], dtype=f4)
    stat_pm, dlbm, bias = _host_weights(
        np.asarray(inputs["Adj_t"], dtype=f4),
        np.asarray(inputs["Adj_s"], dtype=f4),
        np.asarray(inputs["s"], dtype=f4),
        np.asarray(inputs["H"], dtype=f4),
        np.asarray(inputs["W1"], dtype=f4),
        np.asarray(inputs["b1"], dtype=f4),
        np.asarray(inputs["W2"], dtype=f4),
        np.asarray(inputs["b2"], dtype=f4),
    )
    # x pack: [t, a, i, fin] -> [(a,fin), (t,i)]
    xs = []
    for b in range(B):
        xb = x[b].reshape(T, 4, 32, FIN).transpose(1, 3, 0, 2).reshape(128, T * 32)
        xs.append(np.ascontiguousarray(xb.astype(bfp)))
    maps = []
    for c in range(NCORES):
        b, q = c // NQ, c % NQ
        maps.append({
            "xp": xs[b],
            "stat": stat_pm[q],
            "dlb": dlbm,
            "bias": bias,
        })
    return maps


def _unpack_out(res_outb):
    """outb [128=(a,f32), 512=(v,i)] bf16 -> [2048, 16] fp32 for that quarter."""
    o = np.asarray(res_outb).astype(np.float32).reshape(4, 32, 16, 32)  # [a, f, v, i]
    return o.transpose(2, 0, 3, 1).reshape(16 * 128, 32)[:, :FOUT]


def kernel(**inputs) -> np.ndarray:
    import os

    from concourse import bass_utils

    if "nc" not in _CACHE:
        _CACHE["nc"] = _build_nc()
    nc = _CACHE["nc"]

    maps = _in_maps(inputs)
    trace = bool(int(os.environ.get("GTCNN_TRACE", "0")))
    res = bass_utils.run_bass_kernel_spmd(
        nc,
        maps,
        core_ids=list(range(NCORES)),
        trace=trace,
        trace_cores=list(range(NCORES)) if trace else None,
        stitch_traces=False,
    )
    _CACHE["last_results"] = res

    out = np.empty((B, M, FOUT), dtype=np.float32)
    for c in range(NCORES):
        b, q = c // NQ, c % NQ
        out[b, 2048 * q:2048 * (q + 1), :] = _unpack_out(res.results[c]["outb"])
    return out


# revision 24
# speedup vs baseline: 1.1713x; 1.0652x over previous
"""Trainium2 Bass kernel for nn_GTCNN (product-graph GTCNN, 2 layers, K collapsed).

Math (per batch b, x: [M=8192, 32]):
  Adj = s0*I + s1*kron(I_t, As) + s2*kron(At, I_s) + s3*kron(At, As),  T=64, N=128
  h0 = x @ W1 + b1 ; h_{l+1} = tanh((Adj h_l) Heff_l) ; out = h2 @ W2 + b2

v3 device dataflow, per core = (b, t-quarter q); indices n = 32a+i, t = 32c+u, h:
  layouts:
    FD  [part (a,h), free (t,i)]    - feature-contraction matmuls (kron(I4, .))
    NM  [part (a,i), free]          - P/Q n-mix matmuls
    FDT [part (a,u), free (c,i,h)]  - At t-mix matmuls (accumulate over c)
  L1 folds W1 and Heff0 on host:  w = x @ (W1 Heff0)  [FD]
     zpre1 = P w + Q (At w) + rs (x) b~   with b~ = b1 Heff0 and
     rs[(t,n)] = rowsum(P)[n] + rowsum(Q)[n] rowsum(At)[t]  (bias mixed by Adj),
     implemented as a third accumulating matmul with stationary rs.T [64,128]
     and a constant moving delta(t) (x) b~ [64, 2048].
     tanh -> z1 [NM bf16, phys (h,t)]
  L2: P2 native on z1 quarter; A2 z1->FDT; At2 (quarter, padded); C2/D2 small
     fp32 transposes; H1/Q2 run plain fp32; tanh -> z2 [FD quarter]; W2 + b2.
  All big matmuls bf16 (1 cyc/row); PSUM->bf16 casts for the two big transpose
  sources (w, u1) ride the scalar engine. Host packs x / unpacks out, so no
  on-device transposes are spent on I/O.
"""

import numpy as np

T, NS, B, FIN, HID, FOUT = 64, 128, 2, 32, 32, 16
M = T * NS
NCORES, NQ = 8, 4
TQ = T // NQ  # 16 t's per quarter

_CACHE = {}


def _build_nc():
    from contextlib import ExitStack

    import concourse.mybir as mybir
    import concourse.tile as tile
    from concourse import bacc
    from concourse.bass import ds

    f32 = mybir.dt.float32
    bf = mybir.dt.bfloat16
    AF = mybir.ActivationFunctionType

    nc = bacc.Bacc(
        "TRN2",
        target_bir_lowering=False,
        debug=False,
        enable_asserts=False,
        num_devices=NCORES,
    )

    # ---- DRAM I/O ----
    xp = nc.dram_tensor("xp", [128, 2048], bf, kind="ExternalInput")  # [(a,fin),(t,i)]
    # bf16 stationaries, p-major [128, 12*128]; slot s at cols [128s, 128s+128):
    # 0:wti4 (W1@Heff0 kron) 1:B1S (rs.T, rows 0..63) 2:- 3:w2i4 4:P 5:Q
    # 6..9:atb[c][cp] at 6+2c+cp  10,11:atq (per-core quarter)
    stat = nc.dram_tensor("stat", [128, 12 * 128], bf, kind="ExternalInput")
    dlb = nc.dram_tensor("dlb", [64, 2048], bf, kind="ExternalInput")  # delta(t)xb~
    bias = nc.dram_tensor("bias", [128, 1], f32, kind="ExternalInput")  # b2 tiled
    outb = nc.dram_tensor("outb", [128, 512], bf, kind="ExternalOutput")

    H1024 = [slice(1024 * j, 1024 * (j + 1)) for j in range(2)]
    C512 = [slice(512 * j, 512 * (j + 1)) for j in range(4)]

    with tile.TileContext(nc) as tc, ExitStack() as ctx:
        const = ctx.enter_context(tc.tile_pool(name="const", bufs=1))
        st = ctx.enter_context(tc.tile_pool(name="st", bufs=1))
        ps = ctx.enter_context(tc.tile_pool(name="ps", bufs=4, space="PSUM"))

        # ---- constants (scalar HWDGE queue, parallel to x on sync) ----
        stat_f = const.tile([128, 12 * 128], bf, tag="stat")
        nc.scalar.dma_start(stat_f[:], stat.ap())
        dlb_s = const.tile([64, 2048], bf, tag="dlb")
        nc.scalar.dma_start(dlb_s[:], dlb.ap())
        bias_s = const.tile([128, 1], f32, tag="bias")
        nc.scalar.dma_start(bias_s[:], bias.ap())
        stat_s = stat_f[:].rearrange("p (s c) -> p s c", s=12, c=128)
        wti4 = stat_s[:, 0, :]
        b1S = stat_s[0:64, 1, :]
        w2i4 = stat_s[:, 3, :]
        pmat = stat_s[:, 4, :]
        qmat = stat_s[:, 5, :]
        atb = [[stat_s[:, 6 + 2 * c + cp, :] for cp in range(2)] for c in range(2)]
        atq = [stat_s[:, 10 + c, :] for c in range(2)]
        hf1 = stat_s[:, 2, :]

        # ---- x load (packed on host): 4 chunks ----
        x_s = st.tile([128, 2048], bf, tag="x")
        for j in range(4):
            nc.sync.dma_start(x_s[:, C512[j]], xp.ap()[:, C512[j]])

        # =========================== layer 1 ===========================
        # w = x @ (W1 Heff0)   [FD psum, phys (t,i)]
        wp = [ps.tile([128, 1024], f32, tag="ps", name=f"wp{j}") for j in range(2)]
        for j in range(4):
            nc.tensor.matmul(wp[j // 2][:, C512[j % 2]], wti4, x_s[:, C512[j]],
                             start=True, stop=True)
        # cast to bf16 (scalar engine), per c-half; wb phys (c, u, i)
        wb = st.tile([128, 2048], bf, tag="wb")
        for c in range(2):
            nc.scalar.activation(wb[:, H1024[c]], wp[c][:], AF.Identity)

        # D1a: w -> NM   w_nm phys (t, h); contiguous 32-runs on both sides
        w_nm = st.tile([128, 2048], bf, tag="w_nm")
        w_nm_v = w_nm[:].rearrange("p (t h) -> p t h", t=64, h=32)
        wb_v = wb[:].rearrange("p (c u i) -> p c u i", c=2, u=32, i=32)
        for c in range(2):
            nc.vector.transpose(
                out=w_nm_v[:, 32 * c:32 * (c + 1), :], in_=wb_v[:, c]
            )
        # D1b: w -> FDT  w_fdt phys (c, i, h); in iterated (i, u-run str 32)
        w_fdt = st.tile([128, 2048], bf, tag="w_fdt")
        w_fdt_v = w_fdt[:].rearrange("p (c i h) -> p c i h", c=2, i=32, h=32)
        wb_t = wb[:].rearrange("p (c u i) -> p c i u", c=2, u=32, i=32)
        for c in range(2):
            nc.vector.transpose(out=w_fdt_v[:, c], in_=wb_t[:, c])

        # At1: u1[(a,u'), (c', i, h)] accumulated over c; 2 psum tiles by c'
        u1p = [ps.tile([128, 1024], f32, tag="ps", name=f"u1p{j}") for j in range(2)]
        for cp in range(2):
            for c in range(2):
                for k in range(2):
                    nc.tensor.matmul(
                        u1p[cp][:, 512 * k:512 * (k + 1)],
                        atb[c][cp],
                        w_fdt[:, 1024 * c + 512 * k:1024 * c + 512 * (k + 1)],
                        start=(c == 0),
                        stop=(c == 1),
                    )
        # cast u1 to bf16 (scalar engine), per c'
        u1b = st.tile([128, 2048], bf, tag="u1b")
        for cp in range(2):
            nc.scalar.activation(u1b[:, H1024[cp]], u1p[cp][:], AF.Identity)

        # C1: u1 -> NM   u_nm phys (h, c', u'); in per c' iterated (h, i-run)
        u_nm = st.tile([128, 2048], bf, tag="u_nm")
        u_nm_v = u_nm[:].rearrange("p (h c u) -> p c h u", c=2, u=32, h=32)
        u1b_v = u1b[:].rearrange("p (c i h) -> p c h i", c=2, i=32, h=32)
        for cp in range(2):
            nc.vector.transpose(out=u_nm_v[:, cp], in_=u1b_v[:, cp])

        # zpre1 = rs (x) b~  +  P w + Q u   (NM psum, phys (h, t))
        zp1 = [ps.tile([128, 1024], f32, tag="ps", name=f"zp1{j}") for j in range(2)]
        for j in range(4):
            nc.tensor.matmul(zp1[j // 2][:, C512[j % 2]], b1S, dlb_s[:, C512[j]],
                             start=True, stop=False)
        w_nm_s = w_nm[:].rearrange("p (c u h) -> p h c u", c=2, u=32, h=32)
        for j in range(4):
            nc.tensor.matmul(zp1[j // 2][:, C512[j % 2]], pmat,
                             w_nm_s[:, 8 * j:8 * (j + 1)], start=False, stop=False)
        for j in range(4):
            nc.tensor.matmul(zp1[j // 2][:, C512[j % 2]], qmat, u_nm[:, C512[j]],
                             start=False, stop=True)

        # tanh -> z1 [NM bf16, phys (h, t)]
        z1 = st.tile([128, 2048], bf, tag="z1")
        for hh in range(2):
            nc.scalar.activation(z1[:, H1024[hh]], zp1[hh][:], AF.Tanh)

        # =========================== layer 2 ===========================
        # A2: z1 -> FDT  g2 phys (c, h, i); in per c (h, u-run)
        g2 = st.tile([128, 2048], bf, tag="g2")
        g2_v = g2[:].rearrange("p (c h i) -> p c h i", c=2, h=32, i=32)
        z1_v = z1[:].rearrange("p (h c u) -> p c h u", c=2, u=32, h=32)
        for c in range(2):
            nc.vector.transpose(out=g2_v[:, c], in_=z1_v[:, c])

        # At2 (quarter rows, padded to u'32): accumulate over c; moving per c (h,i)
        u2p = ps.tile([128, 1024], f32, tag="ps")
        for c in range(2):
            for k in range(2):
                nc.tensor.matmul(
                    u2p[:, 512 * k:512 * (k + 1)],
                    atq[c],
                    g2[:, 1024 * c + 512 * k:1024 * c + 512 * (k + 1)],
                    start=(c == 0),
                    stop=(c == 1),
                )

        # cast u2 to bf16; C2: u2 -> NM  u2_nm phys (h, u'32)
        u2b = st.tile([128, 1024], bf, tag="u2b")
        nc.scalar.activation(u2b[:], u2p[:], AF.Identity)
        u2_nm = st.tile([128, 1024], bf, tag="u2_nm")
        nc.vector.transpose(
            out=u2_nm[:].rearrange("p (h u) -> p h u", h=32, u=32),
            in_=u2b[:].rearrange("p (h i) -> p h i", h=32, i=32),
        )

        # zpre2 = P z1[quarter] + Q u2  -> psum (v, h) stream
        pid = nc.tensor.partition_id()
        toff = (pid % NQ) * TQ  # t-offset of this core's quarter
        vq = (pid % 2) * TQ  # u'-offset within the padded 32-block
        zp2 = ps.tile([128, 512], f32, tag="ps")
        z1_t = z1[:].rearrange("p (h t) -> p t h", h=32, t=64)
        nc.tensor.matmul(zp2[:], pmat, z1_t[:, ds(toff, TQ), :], start=True, stop=False)
        u2_t = u2_nm[:].rearrange("p (h u) -> p u h", h=32, u=32)
        nc.tensor.matmul(zp2[:], qmat, u2_t[:, ds(vq, TQ), :],
                         start=False, stop=True)

        # cast zpre2 to bf16; D2: -> FD quarter  z2f phys (v, i)
        zp2b = st.tile([128, 512], bf, tag="zp2b")
        nc.scalar.activation(zp2b[:], zp2[:], AF.Identity)
        z2f = st.tile([128, 512], bf, tag="z2f")
        nc.vector.transpose(
            out=z2f[:].rearrange("p (v i) -> p v i", v=16, i=32),
            in_=zp2b[:].rearrange("p (v h) -> p v h", v=16, h=32),
        )

        # pre2 = z2f @ Heff1 ; tanh
        p2p = ps.tile([128, 512], f32, tag="ps")
        nc.tensor.matmul(p2p[:], hf1, z2f[:], start=True, stop=True)
        z2 = st.tile([128, 512], bf, tag="z2")
        nc.scalar.activation(z2[:], p2p[:], AF.Tanh)

        # out = z2 @ W2 + b2  (FD quarter)
        op = ps.tile([128, 512], f32, tag="ps")
        nc.tensor.matmul(op[:], w2i4, z2[:], start=True, stop=True)
        o_s = st.tile([128, 512], bf, tag="o")
        nc.scalar.activation(o_s[:], op[:], AF.Identity, bias=bias_s[:, 0:1])

        nc.sync.dma_start(outb.ap(), o_s[:])

    nc.compile()
    return nc


def _host_weights(Adj_t, Adj_s, s, H, W1, b1, W2, b2):
    import ml_dtypes

    bfp = ml_dtypes.bfloat16
    f4 = np.float32
    I4 = np.eye(4, dtype=f4)
    I128 = np.eye(128, dtype=f4)
    Heff = H.sum(axis=1).astype(f4)  # [2, 32, 32]

    P = (s[0] * I128 + s[1] * Adj_s).astype(f4)
    Q = (s[2] * I128 + s[3] * Adj_s).astype(f4)

    def k4(m):
        return np.kron(I4, m.astype(f4))

    w2pad = np.zeros((32, 32), dtype=f4)
    w2pad[:, :FOUT] = W2

    wt = (W1 @ Heff[0]).astype(f4)  # folded W1*Heff0
    bt = (b1 @ Heff[0]).astype(f4)  # folded bias [32]
    rsP = P.sum(axis=1)  # [128]
    rsQ = Q.sum(axis=1)
    rsAt = Adj_t.sum(axis=1).astype(f4)  # [64]
    rs = rsP[None, :] + rsAt[:, None] * rsQ[None, :]  # [64 t, 128 n]

    stat = np.zeros((12, 128, 128), dtype=f4)
    stat[0] = k4(wt)
    stat[1, :64, :] = rs  # B1S[t2, n]
    stat[2] = k4(Heff[1])
    stat[3] = k4(w2pad)
    stat[4] = P
    stat[5] = Q
    for c in range(2):
        for cp in range(2):
            stat[6 + 2 * c + cp] = k4(Adj_t[32 * c:32 * c + 32, 32 * cp:32 * cp + 32])

    stat_all = np.broadcast_to(stat, (NQ, 12, 128, 128)).copy()
    for q in range(NQ):
        cq, vq = q // 2, (q % 2) * 16
        for c in range(2):
            blk = np.zeros((32, 32), dtype=f4)
            blk[:, vq:vq + 16] = Adj_t[32 * c:32 * c + 32, 32 * cq + vq:32 * cq + vq + 16]
            stat_all[q, 10 + c] = k4(blk)

    # delta(t) x b~ moving const, (h, t)-stream: dlb[t2, h*64 + t] = (t==t2)*bt[h]
    dlbm = np.zeros((64, 32, 64), dtype=f4)
    for t2 in range(64):
        dlbm[t2, :, t2] = bt
    dlbm = dlbm.reshape(64, 2048)

    b2pad = np.zeros(32, dtype=f4)
    b2pad[:FOUT] = b2
    bias = np.ascontiguousarray(np.tile(b2pad, 4)[:, None].astype(f4))

    stat_pm = np.ascontiguousarray(
        stat_all.transpose(0, 2, 1, 3).reshape(NQ, 128, 12 * 128).astype(bfp))
    return stat_pm, np.ascontiguousarray(dlbm.astype(bfp)), bias


def _in_maps(inputs):
    import ml_dtypes

    bfp = ml_dtypes.bfloat16
    f4 = np.float32
    x = np.asarray(inputs["x"], dtype=f4)
    stat_pm, dlbm, bias = _host_weights(
        np.asarray(inputs["Adj_t"], dtype=f4),
        np.asarray(inputs["Adj_s"], dtype=f4),
        np.asarray(inputs["s"], dtype=f4),
        np.asarray(inputs["H"], dtype=f4),
        np.asarray(inputs["W1"], dtype=f4),
        np.asarray(inputs["b1"], dtype=f4),
        np.asarray(inputs["W2"], dtype=f4),
        np.asarray(inputs["b2"], dtype=f4),
    )
    # x pack: [t, a, i, fin] -> [(a,fin), (t,i)]
    xs = []
    for b in range(B):
        xb = x[b].reshape(T, 4, 32, FIN).transpose(1, 3, 0, 2).reshape(128, T * 32)
        xs.append(np.ascontiguousarray(xb.astype(bfp)))
    maps = []
    for c in range(NCORES):
        b, q = c // NQ, c % NQ
        maps.append({
            "xp": xs[b],
            "stat": stat_pm[q],
            "dlb": dlbm,
            "bias": bias,
        })
    return maps


def _unpack_out(res_outb):
    """outb [128=(a,f32), 512=(v,i)] bf16 -> [2048, 16] fp32 for that quarter."""
    o = np.asarray(res_outb).astype(np.float32).reshape(4, 32, 16, 32)  # [a, f, v, i]
    return o.transpose(2, 0, 3, 1).reshape(16 * 128, 32)[:, :FOUT]


def kernel(**inputs) -> np.ndarray:
    import os

    from concourse import bass_utils

    if "nc" not in _CACHE:
        _CACHE["nc"] = _build_nc()
    nc = _CACHE["nc"]

    maps = _in_maps(inputs)
    trace = bool(int(os.environ.get("GTCNN_TRACE", "0")))
    res = bass_utils.run_bass_kernel_spmd(
        nc,
        maps,
        core_ids=list(range(NCORES)),
        trace=trace,
        trace_cores=list(range(NCORES)) if trace else None,
        stitch_traces=False,
    )
    _CACHE["last_results"] = res

    out = np.empty((B, M, FOUT), dtype=np.float32)
    for c in range(NCORES):
        b, q = c // NQ, c % NQ
        out[b, 2048 * q:2048 * (q + 1), :] = _unpack_out(res.results[c]["outb"])
    return out


# revision 25
# speedup vs baseline: 1.2786x; 1.0917x over previous
"""Trainium2 Bass kernel for nn_GTCNN (product-graph GTCNN, 2 layers, K collapsed).

Math (per batch b, x: [M=8192, 32]):
  Adj = s0*I + s1*kron(I_t, As) + s2*kron(At, I_s) + s3*kron(At, As),  T=64, N=128
  h0 = x @ W1 + b1 ; h_{l+1} = tanh((Adj h_l) Heff_l) ; out = h2 @ W2 + b2

v3 device dataflow, per core = (b, t-quarter q); indices n = 32a+i, t = 32c+u, h:
  layouts:
    FD  [part (a,h), free (t,i)]    - feature-contraction matmuls (kron(I4, .))
    NM  [part (a,i), free]          - P/Q n-mix matmuls
    FDT [part (a,u), free (c,i,h)]  - At t-mix matmuls (accumulate over c)
  L1 folds W1 and Heff0 on host:  w = x @ (W1 Heff0)  [FD]
     zpre1 = P w + Q (At w) + rs (x) b~   with b~ = b1 Heff0 and
     rs[(t,n)] = rowsum(P)[n] + rowsum(Q)[n] rowsum(At)[t]  (bias mixed by Adj),
     implemented as a third accumulating matmul with stationary rs.T [64,128]
     and a constant moving delta(t) (x) b~ [64, 2048].
     tanh -> z1 [NM bf16, phys (h,t)]
  L2: P2 native on z1 quarter; A2 z1->FDT; At2 (quarter, padded); C2/D2 small
     fp32 transposes; H1/Q2 run plain fp32; tanh -> z2 [FD quarter]; W2 + b2.
  All big matmuls bf16 (1 cyc/row); PSUM->bf16 casts for the two big transpose
  sources (w, u1) ride the scalar engine. Host packs x / unpacks out, so no
  on-device transposes are spent on I/O.
"""

import numpy as np

T, NS, B, FIN, HID, FOUT = 64, 128, 2, 32, 32, 16
M = T * NS
NCORES, NQ = 8, 4
TQ = T // NQ  # 16 t's per quarter

_CACHE = {}


def _build_nc():
    from contextlib import ExitStack

    import concourse.mybir as mybir
    import concourse.tile as tile
    from concourse import bacc
    from concourse.bass import ds

    f32 = mybir.dt.float32
    bf = mybir.dt.bfloat16
    AF = mybir.ActivationFunctionType

    nc = bacc.Bacc(
        "TRN2",
        target_bir_lowering=False,
        debug=False,
        enable_asserts=False,
        num_devices=NCORES,
    )

    # ---- DRAM I/O ----
    xp = nc.dram_tensor("xp", [128, 2048], bf, kind="ExternalInput")  # [(a,fin),(t,i)]
    # bf16 stationaries, p-major [128, 12*128]; slot s at cols [128s, 128s+128):
    # 0:wti4 (W1@Heff0 kron) 1:B1S (rs.T, rows 0..63) 2:- 3:w2i4 4:P 5:Q
    # 6..9:atb[c][cp] at 6+2c+cp  10,11:atq (per-core quarter)
    stat = nc.dram_tensor("stat", [128, 12 * 128], bf, kind="ExternalInput")
    # bias col0: b~ = b1@Heff0 tiled (applied in the wb cast, FD partition-bias);
    # bias col1: b2 tiled (output bias)
    bias = nc.dram_tensor("bias", [128, 2], f32, kind="ExternalInput")
    outb = nc.dram_tensor("outb", [128, 512], bf, kind="ExternalOutput")

    H1024 = [slice(1024 * j, 1024 * (j + 1)) for j in range(2)]
    C512 = [slice(512 * j, 512 * (j + 1)) for j in range(4)]

    with tile.TileContext(nc) as tc, ExitStack() as ctx:
        const = ctx.enter_context(tc.tile_pool(name="const", bufs=1))
        st = ctx.enter_context(tc.tile_pool(name="st", bufs=1))
        ps = ctx.enter_context(tc.tile_pool(name="ps", bufs=4, space="PSUM"))

        # ---- constants (scalar HWDGE queue, parallel to x on sync) ----
        stat_f = const.tile([128, 12 * 128], bf, tag="stat")
        nc.scalar.dma_start(stat_f[:], stat.ap())
        bias_s = const.tile([128, 2], f32, tag="bias")
        nc.scalar.dma_start(bias_s[:], bias.ap())
        stat_s = stat_f[:].rearrange("p (s c) -> p s c", s=12, c=128)
        wti4 = stat_s[:, 0, :]
        w2i4 = stat_s[:, 3, :]
        pmat = stat_s[:, 4, :]
        qmat = stat_s[:, 5, :]
        atb = [[stat_s[:, 6 + 2 * c + cp, :] for cp in range(2)] for c in range(2)]
        atq = [stat_s[:, 10 + c, :] for c in range(2)]
        hf1 = stat_s[:, 2, :]

        # ---- x load (packed on host): 4 chunks ----
        x_s = st.tile([128, 2048], bf, tag="x")
        for j in range(4):
            nc.sync.dma_start(x_s[:, C512[j]], xp.ap()[:, C512[j]])

        # =========================== layer 1 ===========================
        # w = x @ (W1 Heff0)   [FD psum, phys (t,i)]
        wp = [ps.tile([128, 1024], f32, tag="ps", name=f"wp{j}") for j in range(2)]
        for j in range(4):
            nc.tensor.matmul(wp[j // 2][:, C512[j % 2]], wti4, x_s[:, C512[j]],
                             start=True, stop=True)
        # cast to bf16 + add b~ bias (per-partition in FD); wb phys (c, u, i)
        wb = st.tile([128, 2048], bf, tag="wb")
        for c in range(2):
            nc.scalar.activation(wb[:, H1024[c]], wp[c][:], AF.Identity,
                                 bias=bias_s[:, 0:1])

        # D1a: w -> NM   w_nm phys (t, h); contiguous 32-runs on both sides
        w_nm = st.tile([128, 2048], bf, tag="w_nm")
        w_nm_v = w_nm[:].rearrange("p (t h) -> p t h", t=64, h=32)
        wb_v = wb[:].rearrange("p (c u i) -> p c u i", c=2, u=32, i=32)
        for c in range(2):
            nc.vector.transpose(
                out=w_nm_v[:, 32 * c:32 * (c + 1), :], in_=wb_v[:, c]
            )
        # D1b: w -> FDT  w_fdt phys (c, i, h); in iterated (i, u-run str 32)
        w_fdt = st.tile([128, 2048], bf, tag="w_fdt")
        w_fdt_v = w_fdt[:].rearrange("p (c i h) -> p c i h", c=2, i=32, h=32)
        wb_t = wb[:].rearrange("p (c u i) -> p c i u", c=2, u=32, i=32)
        for c in range(2):
            nc.vector.transpose(out=w_fdt_v[:, c], in_=wb_t[:, c])

        # At1: u1[(a,u'), (c', i, h)] accumulated over c; 2 psum tiles by c'
        u1p = [ps.tile([128, 1024], f32, tag="ps", name=f"u1p{j}") for j in range(2)]
        for cp in range(2):
            for c in range(2):
                for k in range(2):
                    nc.tensor.matmul(
                        u1p[cp][:, 512 * k:512 * (k + 1)],
                        atb[c][cp],
                        w_fdt[:, 1024 * c + 512 * k:1024 * c + 512 * (k + 1)],
                        start=(c == 0),
                        stop=(c == 1),
                    )
        # cast u1 to bf16 (scalar engine), per c'
        u1b = st.tile([128, 2048], bf, tag="u1b")
        for cp in range(2):
            nc.scalar.activation(u1b[:, H1024[cp]], u1p[cp][:], AF.Identity)

        # C1: u1 -> NM   u_nm phys (h, c', u'); in per c' iterated (h, i-run)
        u_nm = st.tile([128, 2048], bf, tag="u_nm")
        u_nm_v = u_nm[:].rearrange("p (h c u) -> p c h u", c=2, u=32, h=32)
        u1b_v = u1b[:].rearrange("p (c i h) -> p c h i", c=2, i=32, h=32)
        for cp in range(2):
            nc.vector.transpose(out=u_nm_v[:, cp], in_=u1b_v[:, cp])

        # zpre1 = P w + Q u   (NM psum, phys (h, t)); bias is already in w
        zp1 = [ps.tile([128, 1024], f32, tag="ps", name=f"zp1{j}") for j in range(2)]
        w_nm_s = w_nm[:].rearrange("p (c u h) -> p h c u", c=2, u=32, h=32)
        for j in range(4):
            nc.tensor.matmul(zp1[j // 2][:, C512[j % 2]], pmat,
                             w_nm_s[:, 8 * j:8 * (j + 1)], start=True, stop=False)
        for j in range(4):
            nc.tensor.matmul(zp1[j // 2][:, C512[j % 2]], qmat, u_nm[:, C512[j]],
                             start=False, stop=True)

        # tanh -> z1 [NM bf16, phys (h, t)]
        z1 = st.tile([128, 2048], bf, tag="z1")
        for hh in range(2):
            nc.scalar.activation(z1[:, H1024[hh]], zp1[hh][:], AF.Tanh)

        # =========================== layer 2 ===========================
        # A2: z1 -> FDT  g2 phys (c, h, i); in per c (h, u-run)
        g2 = st.tile([128, 2048], bf, tag="g2")
        g2_v = g2[:].rearrange("p (c h i) -> p c h i", c=2, h=32, i=32)
        z1_v = z1[:].rearrange("p (h c u) -> p c h u", c=2, u=32, h=32)
        for c in range(2):
            nc.vector.transpose(out=g2_v[:, c], in_=z1_v[:, c])

        # At2 (quarter rows, padded to u'32): accumulate over c; moving per c (h,i)
        u2p = ps.tile([128, 1024], f32, tag="ps")
        for c in range(2):
            for k in range(2):
                nc.tensor.matmul(
                    u2p[:, 512 * k:512 * (k + 1)],
                    atq[c],
                    g2[:, 1024 * c + 512 * k:1024 * c + 512 * (k + 1)],
                    start=(c == 0),
                    stop=(c == 1),
                )

        # cast u2 to bf16; C2: u2 -> NM  u2_nm phys (h, u'32)
        u2b = st.tile([128, 1024], bf, tag="u2b")
        nc.scalar.activation(u2b[:], u2p[:], AF.Identity)
        u2_nm = st.tile([128, 1024], bf, tag="u2_nm")
        nc.vector.transpose(
            out=u2_nm[:].rearrange("p (h u) -> p h u", h=32, u=32),
            in_=u2b[:].rearrange("p (h i) -> p h i", h=32, i=32),
        )

        # zpre2 = P z1[quarter] + Q u2  -> psum (v, h) stream
        pid = nc.tensor.partition_id()
        toff = (pid % NQ) * TQ  # t-offset of this core's quarter
        vq = (pid % 2) * TQ  # u'-offset within the padded 32-block
        zp2 = ps.tile([128, 512], f32, tag="ps")
        z1_t = z1[:].rearrange("p (h t) -> p t h", h=32, t=64)
        nc.tensor.matmul(zp2[:], pmat, z1_t[:, ds(toff, TQ), :], start=True, stop=False)
        u2_t = u2_nm[:].rearrange("p (h u) -> p u h", h=32, u=32)
        nc.tensor.matmul(zp2[:], qmat, u2_t[:, ds(vq, TQ), :],
                         start=False, stop=True)

        # cast zpre2 to bf16; D2: -> FD quarter  z2f phys (v, i)
        zp2b = st.tile([128, 512], bf, tag="zp2b")
        nc.scalar.activation(zp2b[:], zp2[:], AF.Identity)
        z2f = st.tile([128, 512], bf, tag="z2f")
        nc.vector.transpose(
            out=z2f[:].rearrange("p (v i) -> p v i", v=16, i=32),
            in_=zp2b[:].rearrange("p (v h) -> p v h", v=16, h=32),
        )

        # pre2 = z2f @ Heff1 ; tanh
        p2p = ps.tile([128, 512], f32, tag="ps")
        nc.tensor.matmul(p2p[:], hf1, z2f[:], start=True, stop=True)
        z2 = st.tile([128, 512], bf, tag="z2")
        nc.scalar.activation(z2[:], p2p[:], AF.Tanh)

        # out = z2 @ W2 + b2  (FD quarter)
        op = ps.tile([128, 512], f32, tag="ps")
        nc.tensor.matmul(op[:], w2i4, z2[:], start=True, stop=True)
        o_s = st.tile([128, 512], bf, tag="o")
        nc.scalar.activation(o_s[:], op[:], AF.Identity, bias=bias_s[:, 1:2])

        nc.sync.dma_start(outb.ap(), o_s[:])

    nc.compile()
    return nc


def _host_weights(Adj_t, Adj_s, s, H, W1, b1, W2, b2):
    import ml_dtypes

    bfp = ml_dtypes.bfloat16
    f4 = np.float32
    I4 = np.eye(4, dtype=f4)
    I128 = np.eye(128, dtype=f4)
    Heff = H.sum(axis=1).astype(f4)  # [2, 32, 32]

    P = (s[0] * I128 + s[1] * Adj_s).astype(f4)
    Q = (s[2] * I128 + s[3] * Adj_s).astype(f4)

    def k4(m):
        return np.kron(I4, m.astype(f4))

    w2pad = np.zeros((32, 32), dtype=f4)
    w2pad[:, :FOUT] = W2

    wt = (W1 @ Heff[0]).astype(f4)  # folded W1*Heff0
    bt = (b1 @ Heff[0]).astype(f4)  # folded bias [32]

    stat = np.zeros((12, 128, 128), dtype=f4)
    stat[0] = k4(wt)
    stat[2] = k4(Heff[1])
    stat[3] = k4(w2pad)
    stat[4] = P
    stat[5] = Q
    for c in range(2):
        for cp in range(2):
            stat[6 + 2 * c + cp] = k4(Adj_t[32 * c:32 * c + 32, 32 * cp:32 * cp + 32])

    stat_all = np.broadcast_to(stat, (NQ, 12, 128, 128)).copy()
    for q in range(NQ):
        cq, vq = q // 2, (q % 2) * 16
        for c in range(2):
            blk = np.zeros((32, 32), dtype=f4)
            blk[:, vq:vq + 16] = Adj_t[32 * c:32 * c + 32, 32 * cq + vq:32 * cq + vq + 16]
            stat_all[q, 10 + c] = k4(blk)

    b2pad = np.zeros(32, dtype=f4)
    b2pad[:FOUT] = b2
    bias = np.stack([np.tile(bt, 4), np.tile(b2pad, 4)], axis=1).astype(f4)

    stat_pm = np.ascontiguousarray(
        stat_all.transpose(0, 2, 1, 3).reshape(NQ, 128, 12 * 128).astype(bfp))
    return stat_pm, np.ascontiguousarray(bias)


def _in_maps(inputs):
    import ml_dtypes

    bfp = ml_dtypes.bfloat16
    f4 = np.float32
    x = np.asarray(inputs["x"], dtype=f4)
    stat_pm, bias = _host_weights(
        np.asarray(inputs["Adj_t"], dtype=f4),
        np.asarray(inputs["Adj_s"], dtype=f4),
        np.asarray(inputs["s"], dtype=f4),
        np.asarray(inputs["H"], dtype=f4),
        np.asarray(inputs["W1"], dtype=f4),
        np.asarray(inputs["b1"], dtype=f4),
        np.asarray(inputs["W2"], dtype=f4),
        np.asarray(inputs["b2"], dtype=f4),
    )
    # x pack: [t, a, i, fin] -> [(a,fin), (t,i)]
    xs = []
    for b in range(B):
        xb = x[b].reshape(T, 4, 32, FIN).transpose(1, 3, 0, 2).reshape(128, T * 32)
        xs.append(np.ascontiguousarray(xb.astype(bfp)))
    maps = []
    for c in range(NCORES):
        b, q = c // NQ, c % NQ
        maps.append({
            "xp": xs[b],
            "stat": stat_pm[q],
            "bias": bias,
        })
    return maps


def _unpack_out(res_outb):
    """outb [128=(a,f32), 512=(v,i)] bf16 -> [2048, 16] fp32 for that quarter."""
    o = np.asarray(res_outb).astype(np.float32).reshape(4, 32, 16, 32)  # [a, f, v, i]
    return o.transpose(2, 0, 3, 1).reshape(16 * 128, 32)[:, :FOUT]


def kernel(**inputs) -> np.ndarray:
    import os

    from concourse import bass_utils

    if "nc" not in _CACHE:
        _CACHE["nc"] = _build_nc()
    nc = _CACHE["nc"]

    maps = _in_maps(inputs)
    trace = bool(int(os.environ.get("GTCNN_TRACE", "0")))
    res = bass_utils.run_bass_kernel_spmd(
        nc,
        maps,
        core_ids=list(range(NCORES)),
        trace=trace,
        trace_cores=list(range(NCORES)) if trace else None,
        stitch_traces=False,
    )
    _CACHE["last_results"] = res

    out = np.empty((B, M, FOUT), dtype=np.float32)
    for c in range(NCORES):
        b, q = c // NQ, c % NQ
        out[b, 2048 * q:2048 * (q + 1), :] = _unpack_out(res.results[c]["outb"])
    return out
